# revision 1
# baseline (speedup 1.0000x reference)
"""GPT decoder (B=8,T=256,D=512,H=8,L=6,DFF=2048,V=50257) on 8 TRN2 NeuronCores.

Strategy:
- Layers: data-parallel over batch (core c owns batch c). bf16 matmuls with
  fp32 PSUM accumulation; fp32 residual stream, LN and softmax statistics.
- Vocab projection: tensor-parallel over vocab. Final activations are
  transposed locally, AllGathered across the 8 cores (bf16), and each core
  computes all 2048 tokens against its 6284-column shard of Wout.
- Host side only reshapes/casts/shards inputs and unshards the output.
"""
import math
import os

import numpy as np
import ml_dtypes

import concourse.bass as bass
import concourse.tile as tile
from concourse import bacc, mybir
from concourse import bass_utils
from concourse.masks import make_identity

F32 = mybir.dt.float32
BF16 = mybir.dt.bfloat16
I32 = mybir.dt.int32

D = 512
T = 256
H = 8
DK = 64
L = 6
DFF = 2048
V = 50257
B = 8
NCORES = 8
P = 128

VS = 6284           # per-core vocab shard (8 * 6284 = 50272 >= 50257)
VPAD = VS * NCORES
TT = 2              # token tiles per core (T / P)
KB = D // P         # 4 contraction chunks over D
FB = DFF // P       # 16 chunks over DFF
NTILES = [512] * (VS // 512) + ([VS % 512] if VS % 512 else [])  # vocab n-tiles

_CACHE: dict = {}


def _build_program():
    nc = bacc.Bacc("TRN2", target_bir_lowering=False, debug=False,
                   num_devices=NCORES)

    # ---- I/O declarations ------------------------------------------------
    idx_h = nc.dram_tensor("idx", [T, 1], I32, kind="ExternalInput")
    emb_h = nc.dram_tensor("emb", [V, D], BF16, kind="ExternalInput")
    posenc_h = nc.dram_tensor("posenc", [T, D], F32, kind="ExternalInput")
    mask_h = nc.dram_tensor("maskadd", [T, T], F32, kind="ExternalInput")
    # weights pre-tiled on host: partition dim second-to-innermost
    wqkv_h = nc.dram_tensor("wqkv", [L, 3, P, KB, D], BF16, kind="ExternalInput")
    wo_h = nc.dram_tensor("wo", [L, P, KB, D], BF16, kind="ExternalInput")
    w1_h = nc.dram_tensor("w1", [L, P, KB, DFF], BF16, kind="ExternalInput")
    w2_h = nc.dram_tensor("w2", [L, P, FB, D], BF16, kind="ExternalInput")
    b1t_h = nc.dram_tensor("b1t", [L, P, FB], F32, kind="ExternalInput")
    b2_h = nc.dram_tensor("b2", [L, D], F32, kind="ExternalInput")
    ln1g_h = nc.dram_tensor("ln1g", [L, D], F32, kind="ExternalInput")
    ln1b_h = nc.dram_tensor("ln1b", [L, D], F32, kind="ExternalInput")
    ln2g_h = nc.dram_tensor("ln2g", [L, D], F32, kind="ExternalInput")
    ln2b_h = nc.dram_tensor("ln2b", [L, D], F32, kind="ExternalInput")
    lnfg_h = nc.dram_tensor("lnfg", [D], F32, kind="ExternalInput")
    lnfb_h = nc.dram_tensor("lnfb", [D], F32, kind="ExternalInput")
    wout_h = nc.dram_tensor("wout", [P, KB, VS], BF16, kind="ExternalInput")
    bout_h = nc.dram_tensor("bout", [VS], F32, kind="ExternalInput")
    logits_h = nc.dram_tensor("logits", [B * T, VS], F32, kind="ExternalOutput")

    scale = 1.0 / math.sqrt(D)

    def bcast_row(dram_1d_ap, n):
        """DMA-broadcast a [n] DRAM row across all 128 partitions."""
        return bass.AP(tensor=dram_1d_ap.tensor, offset=dram_1d_ap.offset,
                       ap=[[0, P], [1, n]])

    with tile.TileContext(nc) as tc:
        from contextlib import ExitStack
        with ExitStack() as ctx:
            consts = ctx.enter_context(tc.tile_pool(name="consts", bufs=1))
            acts = ctx.enter_context(tc.tile_pool(name="acts", bufs=1))
            scr = ctx.enter_context(tc.tile_pool(name="scr", bufs=3))
            scr2 = ctx.enter_context(tc.tile_pool(name="scr2", bufs=2))
            psB = ctx.enter_context(tc.tile_pool(name="psB", bufs=2, space="PSUM"))
            psM = ctx.enter_context(tc.tile_pool(name="psM", bufs=4, space="PSUM"))
            psT = ctx.enter_context(tc.tile_pool(name="psT", bufs=2, space="PSUM"))
            dram = ctx.enter_context(tc.tile_pool(name="dram", bufs=1, space="DRAM"))

            # ---- constants ----
            ident = consts.tile([P, P], BF16)
            make_identity(nc, ident)
            eps_sb = consts.tile([P, 1], F32)
            nc.vector.memset(eps_sb, 1e-5)
            posenc_sb = consts.tile([P, TT, D], F32)
            nc.sync.dma_start(out=posenc_sb,
                              in_=posenc_h.ap().rearrange("(t p) d -> p t d", p=P))
            mask_sb = consts.tile([P, TT, T], F32)
            nc.sync.dma_start(out=mask_sb,
                              in_=mask_h.ap().rearrange("(t p) s -> p t s", p=P))
            lnfg_sb = consts.tile([P, D], F32)
            nc.sync.dma_start(out=lnfg_sb, in_=bcast_row(lnfg_h.ap(), D))
            lnfb_sb = consts.tile([P, D], F32)
            nc.sync.dma_start(out=lnfb_sb, in_=bcast_row(lnfb_h.ap(), D))

            # ---- persistent activations ----
            x = acts.tile([P, TT, D], F32)          # residual stream
            xn = acts.tile([P, TT, D], BF16)        # post-LN activations
            xnT = acts.tile([P, KB, T], BF16)       # transposed post-LN
            qt = acts.tile([P, KB, T], BF16)        # Q^T (head-major partitions)
            kt = acts.tile([P, KB, T], BF16)        # K^T
            vv = acts.tile([P, TT, D], BF16)        # V natural [t, h*dk]
            ot = acts.tile([P, KB, T], BF16)        # attn out^T
            ht = acts.tile([P, FB, T], BF16)        # FFN hidden^T

            # ---- embedding gather + positional encoding ----
            idx_sb = acts.tile([P, TT], I32)
            nc.sync.dma_start(out=idx_sb,
                              in_=idx_h.ap().rearrange("(t p) one -> p (t one)", p=P))
            for t in range(TT):
                emb_g = scr.tile([P, D], BF16, name="emb_g")
                nc.gpsimd.indirect_dma_start(
                    out=emb_g[:], out_offset=None,
                    in_=emb_h.ap(),
                    in_offset=bass.IndirectOffsetOnAxis(ap=idx_sb[:, t:t + 1], axis=0),
                )
                emb_f = scr.tile([P, D], F32, name="emb_f")
                nc.vector.tensor_copy(out=emb_f, in_=emb_g)
                nc.vector.tensor_add(out=x[:, t], in0=emb_f, in1=posenc_sb[:, t])

            def layernorm(g_rep, b_rep, out_bf):
                """LN over the residual stream x -> out_bf (bf16), fp32 stats."""
                for t in range(TT):
                    stats = scr.tile([P, 6], F32, name="ln_stats")
                    nc.vector.bn_stats(out=stats, in_=x[:, t])
                    mv = scr.tile([P, 2], F32, name="ln_mv")
                    nc.vector.bn_aggr(out=mv, in_=stats)
                    rstd = scr.tile([P, 1], F32, name="ln_rstd")
                    nc.scalar.activation(out=rstd, in_=mv[:, 1:2],
                                         func=mybir.ActivationFunctionType.Sqrt,
                                         bias=eps_sb, scale=1.0)
                    nc.vector.reciprocal(out=rstd, in_=rstd)
                    z = scr.tile([P, D], F32, name="ln_z")
                    nc.vector.tensor_scalar(out=z, in0=x[:, t],
                                            scalar1=mv[:, 0:1], scalar2=rstd,
                                            op0=mybir.AluOpType.subtract,
                                            op1=mybir.AluOpType.mult)
                    nc.vector.tensor_mul(out=z, in0=z, in1=g_rep)
                    nc.vector.tensor_add(out=out_bf[:, t], in0=z, in1=b_rep)

            def transpose_2x4(src_bf, dst):
                """[128, TT, D] token-major -> [128, KB, T] feature-major."""
                for t in range(TT):
                    for kb in range(KB):
                        tp = psT.tile([P, P], BF16, name="pst")
                        nc.tensor.transpose(out=tp[:],
                                            in_=src_bf[:, t, kb * P:(kb + 1) * P],
                                            identity=ident[:])
                        nc.vector.tensor_copy(out=dst[:, kb, t * P:(t + 1) * P],
                                              in_=tp[:])

            # ================= decoder layers =================
            with tc.tile_pool(name="wpool", bufs=2) as wp:
                for l in range(L):
                    # ---- stream this layer's weights ----
                    wqkv_t = wp.tile([P, 3, KB, D], BF16, name="wqkv_t")
                    for m in range(3):
                        nc.sync.dma_start(out=wqkv_t[:, m], in_=wqkv_h.ap()[l, m])
                    wo_t = wp.tile([P, KB, D], BF16, name="wo_t")
                    nc.sync.dma_start(out=wo_t, in_=wo_h.ap()[l])
                    w1_t = wp.tile([P, KB, DFF], BF16, name="w1_t")
                    nc.sync.dma_start(out=w1_t, in_=w1_h.ap()[l])
                    w2_t = wp.tile([P, FB, D], BF16, name="w2_t")
                    nc.sync.dma_start(out=w2_t, in_=w2_h.ap()[l])
                    b1_sb = wp.tile([P, FB], F32, name="b1_sb")
                    nc.sync.dma_start(out=b1_sb, in_=b1t_h.ap()[l])
                    g1 = wp.tile([P, D], F32, name="g1")
                    nc.sync.dma_start(out=g1, in_=bcast_row(ln1g_h.ap()[l], D))
                    bb1 = wp.tile([P, D], F32, name="bb1")
                    nc.sync.dma_start(out=bb1, in_=bcast_row(ln1b_h.ap()[l], D))
                    g2 = wp.tile([P, D], F32, name="g2")
                    nc.sync.dma_start(out=g2, in_=bcast_row(ln2g_h.ap()[l], D))
                    bb2 = wp.tile([P, D], F32, name="bb2")
                    nc.sync.dma_start(out=bb2, in_=bcast_row(ln2b_h.ap()[l], D))
                    b2r = wp.tile([P, D], F32, name="b2r")
                    nc.sync.dma_start(out=b2r, in_=bcast_row(b2_h.ap()[l], D))

                    # ---- LN1 + transpose ----
                    layernorm(g1, bb1, xn)
                    transpose_2x4(xn, xnT)

                    # ---- Q^T, K^T (head-pair-major), V natural ----
                    for m, dst in ((0, qt), (1, kt)):
                        for pair in range(KB):
                            ps = psM.tile([P, T], F32, name="psm")
                            for kb in range(KB):
                                nc.tensor.matmul(
                                    ps[:],
                                    wqkv_t[:, m, kb, pair * P:(pair + 1) * P],
                                    xnT[:, kb],
                                    start=(kb == 0), stop=(kb == KB - 1))
                            nc.vector.tensor_copy(out=dst[:, pair], in_=ps[:])
                    for t in range(TT):
                        ps = psB.tile([P, D], F32, name="psb")
                        for kb in range(KB):
                            nc.tensor.matmul(ps[:], xnT[:, kb, t * P:(t + 1) * P],
                                             wqkv_t[:, 2, kb],
                                             start=(kb == 0), stop=(kb == KB - 1))
                        nc.vector.tensor_copy(out=vv[:, t], in_=ps[:])

                    # ---- attention, one head at a time ----
                    for pair in range(KB):
                        ot_ps = psM.tile([P, T], F32, name="psm")
                        for sub in range(2):
                            h = pair * 2 + sub
                            off = sub * DK
                            at = scr2.tile([P, TT, T], BF16, name="at")
                            for tq in range(TT):
                                s_ps = psM.tile([P, T], F32, name="psm")
                                nc.tensor.matmul(
                                    s_ps[:],
                                    qt[off:off + DK, pair, tq * P:(tq + 1) * P],
                                    kt[off:off + DK, pair],
                                    start=True, stop=True)
                                sm = scr.tile([P, T], F32, name="sm")
                                nc.vector.tensor_add(out=sm, in0=s_ps,
                                                     in1=mask_sb[:, tq])
                                pexp = scr.tile([P, T], F32, name="pexp")
                                den = scr.tile([P, 1], F32, name="den")
                                nc.scalar.activation(
                                    out=pexp, in_=sm,
                                    func=mybir.ActivationFunctionType.Exp,
                                    scale=scale, accum_out=den)
                                nc.vector.reciprocal(out=den, in_=den)
                                a_bf = scr.tile([P, T], BF16, name="a_bf")
                                nc.vector.tensor_scalar_mul(out=a_bf, in0=pexp,
                                                            scalar1=den)
                                for tk in range(TT):
                                    tp = psT.tile([P, P], BF16, name="pst")
                                    nc.tensor.transpose(
                                        out=tp[:],
                                        in_=a_bf[:, tk * P:(tk + 1) * P],
                                        identity=ident[:])
                                    nc.vector.tensor_copy(
                                        out=at[:, tk, tq * P:(tq + 1) * P],
                                        in_=tp[:])
                            # O^T (this head) = V_h^T @ A^T
                            for tk in range(TT):
                                nc.tensor.matmul(
                                    ot_ps[off:off + DK, :],
                                    vv[:, tk, h * DK:(h + 1) * DK],
                                    at[:, tk],
                                    start=(tk == 0), stop=(tk == TT - 1))
                        nc.vector.tensor_copy(out=ot[:, pair], in_=ot_ps[:])

                    # ---- x += O @ Wo ----
                    for tq in range(TT):
                        ps = psB.tile([P, D], F32, name="psb")
                        for kb in range(KB):
                            nc.tensor.matmul(ps[:], ot[:, kb, tq * P:(tq + 1) * P],
                                             wo_t[:, kb],
                                             start=(kb == 0), stop=(kb == KB - 1))
                        nc.vector.tensor_add(out=x[:, tq], in0=x[:, tq], in1=ps[:])

                    # ---- LN2 + FFN ----
                    layernorm(g2, bb2, xn)
                    transpose_2x4(xn, xnT)
                    for fc in range(FB):
                        ps = psM.tile([P, T], F32, name="psm")
                        for kb in range(KB):
                            nc.tensor.matmul(ps[:],
                                             w1_t[:, kb, fc * P:(fc + 1) * P],
                                             xnT[:, kb],
                                             start=(kb == 0), stop=(kb == KB - 1))
                        nc.scalar.activation(out=ht[:, fc], in_=ps[:],
                                             func=mybir.ActivationFunctionType.Relu,
                                             bias=b1_sb[:, fc:fc + 1], scale=1.0)
                    for tq in range(TT):
                        ps = psB.tile([P, D], F32, name="psb")
                        for fc in range(FB):
                            nc.tensor.matmul(ps[:], ht[:, fc, tq * P:(tq + 1) * P],
                                             w2_t[:, fc],
                                             start=(fc == 0), stop=(fc == FB - 1))
                        nc.vector.tensor_add(out=x[:, tq], in0=x[:, tq], in1=ps[:])
                        nc.vector.tensor_add(out=x[:, tq], in0=x[:, tq], in1=b2r)

            # ================= final LN + all-gather =================
            layernorm(lnfg_sb, lnfb_sb, xn)
            transpose_2x4(xn, xnT)
            ag_in = dram.tile([D, T], BF16)
            ag_out = dram.tile([NCORES * D, T], BF16, addr_space="Shared")
            for kb in range(KB):
                nc.sync.dma_start(out=ag_in[kb * P:(kb + 1) * P, :], in_=xnT[:, kb])
            nc.gpsimd.collective_compute(
                "AllGather", mybir.AluOpType.bypass,
                replica_groups=[list(range(NCORES))],
                ins=[ag_in[:]], outs=[ag_out[:]])

            # ================= vocab projection =================
            with tc.tile_pool(name="vpool", bufs=1) as vp, \
                 tc.tile_pool(name="vstream", bufs=3) as vs, \
                 tc.tile_pool(name="vout", bufs=4) as vo:
                xg = vp.tile([P, KB, B * T], BF16)
                for b in range(B):
                    for kb in range(KB):
                        nc.sync.dma_start(
                            out=xg[:, kb, b * T:(b + 1) * T],
                            in_=ag_out[b * D + kb * P: b * D + (kb + 1) * P, :])
                n0 = 0
                for nsz in NTILES:
                    wout_t = vs.tile([P, KB, 512], BF16, name="wout_t")
                    nc.sync.dma_start(out=wout_t[:, :, :nsz],
                                      in_=wout_h.ap()[:, :, n0:n0 + nsz])
                    boutr = vs.tile([P, 512], F32, name="boutr")
                    nc.sync.dma_start(out=boutr[:, :nsz],
                                      in_=bcast_row(bout_h.ap()[n0:n0 + nsz], nsz))
                    for tq in range(B * T // P):
                        ps = psB.tile([P, 512], F32, name="psb")
                        for kb in range(KB):
                            nc.tensor.matmul(ps[:, :nsz],
                                             xg[:, kb, tq * P:(tq + 1) * P],
                                             wout_t[:, kb, :nsz],
                                             start=(kb == 0), stop=(kb == KB - 1))
                        lg = vo.tile([P, 512], F32, name="lg")
                        nc.vector.tensor_add(out=lg[:, :nsz], in0=ps[:, :nsz],
                                             in1=boutr[:, :nsz])
                        nc.sync.dma_start(
                            out=logits_h.ap()[tq * P:(tq + 1) * P, n0:n0 + nsz],
                            in_=lg[:, :nsz])
                    n0 += nsz

    nc.compile()
    return nc


def _prep_inputs(inputs):
    """Host-side shard/cast/layout. Returns per-core input maps."""
    f32 = np.float32
    bf16 = ml_dtypes.bfloat16

    idx = np.asarray(inputs["idx"])
    emb = np.asarray(inputs["emb"], f32)

    # positional encoding (input-independent constant)
    pos = np.arange(T, dtype=np.float64)[:, None]
    div = np.exp(np.arange(0, D, 2, dtype=np.float64) * (-math.log(10000.0) / D))
    pe = np.zeros((T, D), f32)
    pe[:, 0::2] = np.sin(pos * div).astype(f32)
    pe[:, 1::2] = np.cos(pos * div).astype(f32)

    maskadd = np.where(np.tril(np.ones((T, T), bool)), 0.0, -1e9).astype(f32)

    wq = np.asarray(inputs["Wq"], f32)  # [L, H, D, DK]
    wk = np.asarray(inputs["Wk"], f32)
    wv = np.asarray(inputs["Wv"], f32)
    # [L, 3, D, H*DK] -> pre-tiled [L, 3, P, KB, D]
    wqkv = np.stack([
        wq.transpose(0, 2, 1, 3).reshape(L, D, D),
        wk.transpose(0, 2, 1, 3).reshape(L, D, D),
        wv.transpose(0, 2, 1, 3).reshape(L, D, D),
    ], axis=1)
    wqkv_t = np.ascontiguousarray(
        wqkv.reshape(L, 3, KB, P, D).transpose(0, 1, 3, 2, 4)).astype(bf16)
    wo_t = np.ascontiguousarray(
        np.asarray(inputs["Wo"], f32).reshape(L, KB, P, D)
        .transpose(0, 2, 1, 3)).astype(bf16)
    w1_t = np.ascontiguousarray(
        np.asarray(inputs["W1"], f32).reshape(L, KB, P, DFF)
        .transpose(0, 2, 1, 3)).astype(bf16)
    w2_t = np.ascontiguousarray(
        np.asarray(inputs["W2"], f32).reshape(L, FB, P, D)
        .transpose(0, 2, 1, 3)).astype(bf16)
    b1t = np.ascontiguousarray(
        np.asarray(inputs["b1"], f32).reshape(L, FB, P).transpose(0, 2, 1))

    wout = np.asarray(inputs["Wout"], f32)
    bout = np.asarray(inputs["bout"], f32)
    wout_pad = np.zeros((D, VPAD), f32)
    wout_pad[:, :V] = wout
    bout_pad = np.zeros((VPAD,), f32)
    bout_pad[:V] = bout

    emb_bf = emb.astype(bf16)

    common = dict(
        emb=emb_bf, posenc=pe, maskadd=maskadd,
        wqkv=wqkv_t, wo=wo_t, w1=w1_t, w2=w2_t, b1t=b1t,
        b2=np.asarray(inputs["b2"], f32),
        ln1g=np.asarray(inputs["ln1_g"], f32), ln1b=np.asarray(inputs["ln1_b"], f32),
        ln2g=np.asarray(inputs["ln2_g"], f32), ln2b=np.asarray(inputs["ln2_b"], f32),
        lnfg=np.asarray(inputs["lnf_g"], f32), lnfb=np.asarray(inputs["lnf_b"], f32),
    )
    in_maps = []
    for c in range(NCORES):
        m = dict(common)
        m["idx"] = np.ascontiguousarray(idx[c].astype(np.int32).reshape(T, 1))
        ws = wout_pad[:, c * VS:(c + 1) * VS]
        m["wout"] = np.ascontiguousarray(
            ws.reshape(KB, P, VS).transpose(1, 0, 2)).astype(bf16)
        m["bout"] = np.ascontiguousarray(bout_pad[c * VS:(c + 1) * VS])
        in_maps.append(m)
    return in_maps


def _unshard(results):
    shards = [results[c]["logits"] for c in range(NCORES)]  # each [B*T, VS]
    full = np.concatenate(shards, axis=1)[:, :V]            # [B*T, V]
    return np.ascontiguousarray(full.reshape(B, T, V))


def kernel(**inputs):
    if "nc" not in _CACHE:
        _CACHE["nc"] = _build_program()
    nc = _CACHE["nc"]
    in_maps = _prep_inputs(inputs)

    if os.environ.get("KERNEL_USE_SIM"):
        from concourse.bass_interp import MultiCoreSim
        sim = MultiCoreSim(nc, num_cores=NCORES,
                           num_workers=int(os.environ.get("KERNEL_SIM_WORKERS", "8")))
        for c in range(NCORES):
            for name, val in in_maps[c].items():
                sim.cores[c].tensor(name)[:] = val
        sim.simulate()
        results = [
            {"logits": np.array(sim.cores[c].tensor("logits"))}
            for c in range(NCORES)
        ]
        return _unshard(results)

    res = bass_utils.run_bass_kernel_spmd(
        nc, in_maps, core_ids=list(range(NCORES)))
    return _unshard(res.results)

